# revision 4
# baseline (speedup 1.0000x reference)
"""Distributed causal-attention kernel for 8 TRN2 NeuronCores.

Problem: B=4, S=1024, D=1024, H=16 causal attention block returning
(a, w) where a = proj(attn output) and w = softmax attention probs.

Sharding (tensor-parallel heads x data-parallel batch):
  core c -> (batch b = c//2, head-group g = c%2) ; each group = 8 heads.
  Each core computes its group's QKV projection, causal softmax
  (writing its [8, S, S] slice of w), A@V, and a partial output
  projection a_part = attn_g @ w_proj[rows g].  Host sums the two
  partials per batch (a = part0 + part1 + b_proj + b_attn_v @ w_proj).
  No device collectives needed.

Compute is bf16 on the TensorEngine (f32 PSUM accumulation); softmax
exp runs in f32 on the ScalarEngine.  The 1/sqrt(HD)=0.125 score scale
is folded into the Q weights on the host (exact, power of two).
Scores are computed in both [q,k] (for the softmax/w output) and
[k,q] (for the A@V matmul) orientations -- cheaper on TRN2 than any
transpose path.  Upper-triangular w stays zero: output DRAM buffers
are zero-initialized by the runtime, so the masked region is never
computed or written.
"""

import numpy as np
import ml_dtypes

B, S, D, H = 4, 1024, 1024, 16
HD = D // H          # 64 head dim
G = H // 2           # 8 heads per core
GD = G * HD          # 512 dims per head group
NB = S // 128        # 8 token blocks
DK = D // 128        # 8 contraction tiles
FT = GD // 128       # 4 feature tiles per group
NCORES = 8
NEG = 10000.0

_CACHE = {}


def _build_bass():
    from contextlib import ExitStack
    import concourse.bass as bass
    import concourse.tile as tile
    from concourse import bacc, mybir
    from concourse.masks import make_identity

    f32 = mybir.dt.float32
    bf16 = mybir.dt.bfloat16
    ts = bass.ts
    Exp = mybir.ActivationFunctionType.Exp

    nc = bacc.Bacc(
        "TRN2",
        target_bir_lowering=False,
        debug=False,
        enable_asserts=False,
        num_devices=NCORES,
    )

    xT = nc.dram_tensor("xT", [D, S], bf16, kind="ExternalInput").ap()
    qw = nc.dram_tensor("qw", [D, GD], bf16, kind="ExternalInput").ap()
    kw = nc.dram_tensor("kw", [D, GD], bf16, kind="ExternalInput").ap()
    vw = nc.dram_tensor("vw", [D, GD], bf16, kind="ExternalInput").ap()
    qbias = nc.dram_tensor("qbias", [GD, 1], f32, kind="ExternalInput").ap()
    kbias = nc.dram_tensor("kbias", [GD, 1], f32, kind="ExternalInput").ap()
    wp = nc.dram_tensor("wp", [GD, D], bf16, kind="ExternalInput").ap()
    w_out = nc.dram_tensor("w_out", [G, S, S], bf16, kind="ExternalOutput").ap()
    a_out = nc.dram_tensor("a_out", [S, D], f32, kind="ExternalOutput").ap()

    with tile.TileContext(nc) as tc, ExitStack() as ctx:
        const = ctx.enter_context(tc.tile_pool(name="const", bufs=1))
        work = ctx.enter_context(tc.tile_pool(name="work", bufs=3))
        small = ctx.enter_context(tc.tile_pool(name="small", bufs=4))
        psum = ctx.enter_context(tc.tile_pool(name="psum", bufs=2, space="PSUM"))

        # ---- persistent SBUF tensors ----
        xT_sb = [const.tile([128, S], bf16, tag=f"xT{i}", name=f"xT{i}") for i in range(DK)]
        qw_sb = [const.tile([128, GD], bf16, tag=f"qw{i}", name=f"qw{i}") for i in range(DK)]
        kw_sb = [const.tile([128, GD], bf16, tag=f"kw{i}", name=f"kw{i}") for i in range(DK)]
        vw_sb = [const.tile([128, GD], bf16, tag=f"vw{i}", name=f"vw{i}") for i in range(DK)]
        wp_sb = [const.tile([128, D], bf16, tag=f"wp{i}", name=f"wp{i}") for i in range(FT)]
        qb_sb = [const.tile([128, 1], f32, tag=f"qb{i}", name=f"qb{i}") for i in range(FT)]
        kb_sb = [const.tile([128, 1], f32, tag=f"kb{i}", name=f"kb{i}") for i in range(FT)]
        qT_sb = [const.tile([128, S], bf16, tag=f"qT{i}", name=f"qT{i}") for i in range(FT)]
        kT_sb = [const.tile([128, S], bf16, tag=f"kT{i}", name=f"kT{i}") for i in range(FT)]
        v_sb = [const.tile([128, GD], bf16, tag=f"v{i}", name=f"v{i}") for i in range(NB)]
        attn_sb = [const.tile([128, GD], bf16, tag=f"at{i}", name=f"at{i}") for i in range(NB)]
        attnT_sb = [const.tile([128, S], bf16, tag=f"aT{i}", name=f"aT{i}") for i in range(FT)]
        mask_lo = const.tile([128, 128], f32, tag="mlo")
        mask_up = const.tile([128, 128], f32, tag="mup")
        ident = const.tile([128, 128], bf16, tag="ident")

        for dk in range(DK):
            nc.sync.dma_start(xT_sb[dk][:], xT[ts(dk, 128), :])
            nc.sync.dma_start(qw_sb[dk][:], qw[ts(dk, 128), :])
            nc.sync.dma_start(kw_sb[dk][:], kw[ts(dk, 128), :])
            nc.sync.dma_start(vw_sb[dk][:], vw[ts(dk, 128), :])
        for ft in range(FT):
            nc.sync.dma_start(wp_sb[ft][:], wp[ts(ft, 128), :])
            nc.sync.dma_start(qb_sb[ft][:], qbias[ts(ft, 128), :])
            nc.sync.dma_start(kb_sb[ft][:], kbias[ts(ft, 128), :])

        # additive causal masks for the diagonal score blocks:
        # mask_lo[q, k] = 0 if k <= q else -NEG      ([q, k] orientation)
        # mask_up[k, q] = 0 if q >= k else -NEG      ([k, q] orientation)
        nc.gpsimd.memset(mask_lo[:], 0.0)
        nc.gpsimd.affine_select(
            out=mask_lo[:], in_=mask_lo[:],
            compare_op=mybir.AluOpType.is_ge, fill=-NEG,
            base=0, pattern=[[-1, 128]], channel_multiplier=1,
        )
        nc.gpsimd.memset(mask_up[:], 0.0)
        nc.gpsimd.affine_select(
            out=mask_up[:], in_=mask_up[:],
            compare_op=mybir.AluOpType.is_ge, fill=-NEG,
            base=0, pattern=[[1, 128]], channel_multiplier=-1,
        )
        make_identity(nc, ident[:])

        # ---- QKV projections ----
        # qT/kT in [feature, token] layout: psum = w.T @ xT
        for ft in range(FT):
            for nh in range(2):
                ps = psum.tile([128, 512], f32, tag="ps_t")
                for dk in range(DK):
                    nc.tensor.matmul(
                        ps[:], qw_sb[dk][:, ts(ft, 128)], xT_sb[dk][:, ts(nh, 512)],
                        start=(dk == 0), stop=(dk == DK - 1),
                    )
                nc.scalar.add(qT_sb[ft][:, ts(nh, 512)], ps[:], qb_sb[ft][:])
            for nh in range(2):
                ps = psum.tile([128, 512], f32, tag="ps_t")
                for dk in range(DK):
                    nc.tensor.matmul(
                        ps[:], kw_sb[dk][:, ts(ft, 128)], xT_sb[dk][:, ts(nh, 512)],
                        start=(dk == 0), stop=(dk == DK - 1),
                    )
                nc.scalar.add(kT_sb[ft][:, ts(nh, 512)], ps[:], kb_sb[ft][:])
        # v in [token, feature] layout: psum = xT.T @ vw   (v bias folded on host)
        for tt in range(NB):
            ps = psum.tile([128, 512], f32, tag="ps_t")
            for dk in range(DK):
                nc.tensor.matmul(
                    ps[:], xT_sb[dk][:, ts(tt, 128)], vw_sb[dk][:],
                    start=(dk == 0), stop=(dk == DK - 1),
                )
            nc.vector.tensor_copy(v_sb[tt][:], ps[:])

        # ---- attention per (head, q-block) ----
        for h in range(G):
            fq = h // 2
            po = 64 * (h % 2)
            for qb in range(NB):
                W = (qb + 1) * 128
                qsl = qT_sb[fq][po:po + 64, ts(qb, 128)]

                # scores row s[q, k], k in [0, W)
                ps_s = psum.tile([128, 1024], f32, tag="ps_wide")
                for c0 in range(0, W, 512):
                    cw = min(512, W - c0)
                    nc.tensor.matmul(
                        ps_s[:, c0:c0 + cw], qsl,
                        kT_sb[fq][po:po + 64, c0:c0 + cw],
                        start=True, stop=True,
                    )
                nc.vector.tensor_add(
                    ps_s[:, ts(qb, 128)], ps_s[:, ts(qb, 128)], mask_lo[:]
                )
                E = work.tile([128, S], bf16, tag="E")
                rs = small.tile([128, 1], f32, tag="rs")
                nc.scalar.activation(E[:, :W], ps_s[:, :W], Exp, accum_out=rs[:])
                rcp = small.tile([128, 1], f32, tag="rcp")
                nc.vector.reciprocal(rcp[:], rs[:])
                wrow = work.tile([128, S], bf16, tag="wrow")
                nc.vector.tensor_scalar_mul(wrow[:, :W], E[:, :W], rcp[:])
                nc.sync.dma_start(w_out[h, ts(qb, 128), 0:W], wrow[:, :W])

                # transposed scores sT[k, q] -> exp -> ET (unnormalized)
                ET = work.tile([128, S], bf16, tag="ET")
                for g0 in range(0, qb + 1, 4):
                    gn = min(4, qb + 1 - g0)
                    ps_t = psum.tile([128, 512], f32, tag="ps_t")
                    for j in range(gn):
                        kb = g0 + j
                        nc.tensor.matmul(
                            ps_t[:, ts(j, 128)],
                            kT_sb[fq][po:po + 64, ts(kb, 128)], qsl,
                            start=True, stop=True,
                        )
                        if kb == qb:
                            nc.vector.tensor_add(
                                ps_t[:, ts(j, 128)], ps_t[:, ts(j, 128)], mask_up[:]
                            )
                    nc.scalar.activation(
                        ET[:, g0 * 128:(g0 + gn) * 128], ps_t[:, :gn * 128], Exp
                    )

                # a[q, d] = (ET.T @ v) * rcp
                ps_a = psum.tile([128, 128], f32, tag="ps_a")
                for kb in range(qb + 1):
                    nc.tensor.matmul(
                        ps_a[:, 0:64], ET[:, ts(kb, 128)],
                        v_sb[kb][:, HD * h:HD * h + 64],
                        start=(kb == 0), stop=(kb == qb),
                    )
                nc.vector.tensor_scalar_mul(
                    attn_sb[qb][:, HD * h:HD * h + 64], ps_a[:, 0:64], rcp[:]
                )

        # ---- transpose attn -> [feature, token] via PE ----
        for ft in range(FT):
            for tt in range(NB):
                ps_tr = psum.tile([128, 128], f32, tag="ps_a")
                nc.tensor.matmul(
                    ps_tr[:], attn_sb[tt][:, ts(ft, 128)], ident[:],
                    start=True, stop=True,
                )
                nc.vector.tensor_copy(attnT_sb[ft][:, ts(tt, 128)], ps_tr[:])

        # ---- output projection partial: a_part = attn_g @ wp_g ----
        for tt in range(NB):
            ps_p = psum.tile([128, 1024], f32, tag="ps_wide")
            for nh in range(2):
                for ft in range(FT):
                    nc.tensor.matmul(
                        ps_p[:, ts(nh, 512)], attnT_sb[ft][:, ts(tt, 128)],
                        wp_sb[ft][:, ts(nh, 512)],
                        start=(ft == 0), stop=(ft == FT - 1),
                    )
            ao = work.tile([128, 1024], f32, tag="aout")
            nc.vector.tensor_copy(ao[:], ps_p[:])
            nc.sync.dma_start(a_out[ts(tt, 128), :], ao[:])

    nc.compile()
    return nc


def _numpy_reference(x, mask, w_attn, b_attn, w_proj, b_proj):
    # Defensive fallback for a nonzero padding mask (the problem spec fills
    # it with zeros); replicates the reference math in f32 numpy.
    b, s, d = x.shape
    qkv = x @ w_attn + b_attn
    q, k, v = np.split(qkv, 3, axis=2)

    def split_heads(t):
        return t.reshape(b, s, H, HD).transpose(0, 2, 1, 3)

    q, k, v = split_heads(q), split_heads(k), split_heads(v)
    w = np.einsum("bhqd,bhkd->bhqk", q, k) / np.sqrt(np.float32(HD))
    i = np.arange(s)[:, None]
    j = np.arange(s)[None, :]
    bmask = (i >= j).astype(w.dtype)[None, None]
    w = w * bmask - NEG * (1.0 - bmask)
    w = w + mask
    w = w - w.max(axis=-1, keepdims=True)
    e = np.exp(w)
    w = e / e.sum(axis=-1, keepdims=True)
    a = np.einsum("bhqk,bhkd->bhqd", w, v)
    a = a.transpose(0, 2, 1, 3).reshape(b, s, d)
    a = a @ w_proj + b_proj
    return a.astype(np.float32), w.astype(np.float32)


def kernel(**inputs):
    x = np.asarray(inputs["x"], dtype=np.float32)
    mask = np.asarray(inputs["mask"], dtype=np.float32)
    w_attn = np.asarray(inputs["w_attn"], dtype=np.float32)
    b_attn = np.asarray(inputs["b_attn"], dtype=np.float32)
    w_proj = np.asarray(inputs["w_proj"], dtype=np.float32)
    b_proj = np.asarray(inputs["b_proj"], dtype=np.float32)

    if np.any(mask != 0.0):
        return _numpy_reference(x, mask, w_attn, b_attn, w_proj, b_proj)

    from concourse.bass_utils import run_bass_kernel_spmd

    if "nc" not in _CACHE:
        _CACHE["nc"] = _build_bass()
    nc = _CACHE["nc"]

    bf = ml_dtypes.bfloat16
    in_maps = []
    for c in range(NCORES):
        b, g = divmod(c, 2)
        sl = slice(GD * g, GD * (g + 1))
        in_maps.append({
            "xT": np.ascontiguousarray(x[b].T).astype(bf),
            "qw": np.ascontiguousarray(w_attn[:, sl] * 0.125).astype(bf),
            "kw": np.ascontiguousarray(w_attn[:, D + GD * g:D + GD * (g + 1)]).astype(bf),
            "vw": np.ascontiguousarray(w_attn[:, 2 * D + GD * g:2 * D + GD * (g + 1)]).astype(bf),
            "qbias": np.ascontiguousarray(b_attn[0, sl] * 0.125).reshape(GD, 1).astype(np.float32),
            "kbias": np.ascontiguousarray(b_attn[0, D + GD * g:D + GD * (g + 1)]).reshape(GD, 1).astype(np.float32),
            "wp": np.ascontiguousarray(w_proj[sl, :]).astype(bf),
        })

    res = run_bass_kernel_spmd(nc, in_maps, core_ids=list(range(NCORES))).results

    w = np.empty((B, H, S, S), np.float32)
    a = np.empty((B, S, D), np.float32)
    for c in range(NCORES):
        b, g = divmod(c, 2)
        w[b, G * g:G * (g + 1)] = res[c]["w_out"].astype(np.float32)
    for b in range(B):
        a[b] = res[2 * b]["a_out"] + res[2 * b + 1]["a_out"]
    # exact host-side bias folds: v-bias contributes (sum_k w = 1) b_v @ w_proj
    a += (b_proj[0] + b_attn[0, 2 * D:] @ w_proj).reshape(1, 1, D)
    return a, w
